# revision 17
# baseline (speedup 1.0000x reference)
"""Grouped 2-layer MLP (ConvNN) Trainium2 kernel.

Math (per group g of SIZE=2048):
    h[b,g,:]   = LeakyReLU_0.2(W0[g] @ x[b] + b0[g])     (64 -> 64)
    out[b,g,:] = W1[g] @ h[b,g,:] + b1[g]                (64 -> 64)

Measured on the target axon-tunneled TRN2: 181.7us HW exec (baseline
was 500.3us), rel err 3.2e-3 (gate 2e-2).  Per-core engine busy at the
final shape: DVE 145.6us / ACT 143.8us (the joint PSUM-evacuation
floor — TRN2 matmul output must be fp32, so every one of the 33.6M
evacuated elements per core crosses PSUM->SBUF at 1 elem/lane/cycle),
PE ~137us, DMA ~108us.

Strategy (row-tiled L0, evacuation-balanced pipeline):
  - Shard the group axis over 8 cores (256 groups/core = 128 pairs of
    groups), fully independent, no collectives.
  - Per pair t the dataflow is
        mm0 (PE) -> hps (PSUM fp32) -> Prelu+b0 (ACT) -> hsb (SBUF bf16)
        mm1 (PE) -> ops (PSUM fp32) -> +b1 (DVE)      -> osb (SBUF f16) -> DMA
    With FD=1024 per-pair evacuation ops: ACT ~1.05us, DVE ~1.19us; 8 of
    128 pairs route the layer-1 bias-add to ACT so both engines sit at
    ~144us.  PSUM: 4 rotating [128,1024] fp32 tiles (2 banks) = all 8
    banks, hps/ops double-buffered.
  - Layer-0 runs TWO pairs concurrently as PE row-tiles: even pair on
    array rows 0-63, odd pair on rows 64-127 (x duplicated on both
    partition halves, per-pair W0 stationaries stacked likewise).  Each
    K=64 stream uses half the 256B/cycle moving bus, so the two streams
    coexist and L0 time halves.  The denser activity also keeps the HAM
    clock-gate warm (2.4GHz) for the whole run — without row-tiling this
    environment's PE stayed throttled at 1.2GHz (427ns vs 216ns per
    N=512 matmul) despite 99% occupancy.
  - Layer-1 stationary is a host-built 128x128 block-diagonal (2 groups
    per pair, K=128 uses the full bus); off-diagonal zeros kill cross
    terms and cost nothing since matmul time is N-driven.
  - PE instruction stream is software-pipelined (mm0 of pairs 2u,2u+1
    emitted before mm1 of pairs 2u-2,2u-1) so the PE never waits on ACT.
  - Weights are host-packed exactly in stationary layout and streamed in
    chunks (small first chunk for a fast start) ~32 pairs ahead of use:
    startup-critical loads on the scalar HWDGE ring, bulk on the gpsimd
    SWDGE ring, leaving the sync ring exclusively for the output stream.
  - bf16 matmul operands (fp8 fails the 2e-2 gate: even single-tensor
    e4m3 variants measure 1.9-3.5e-2), f16 output (halves write traffic)
    widened to fp32 on host.
"""

from contextlib import ExitStack

import numpy as np
from ml_dtypes import bfloat16

import concourse.bass as bass
import concourse.mybir as mybir
import concourse.tile as tile
from concourse.bass_utils import run_bass_kernel_spmd

B = 1024
IN_DIM = 64
SIZE = 2048
D1 = 64
D2 = 64
NEG_SLOPE = 0.2
N_CORES = 8
GPC = SIZE // N_CORES  # 256 groups per core
NPAIR = GPC // 2  # 128 group-pairs per core
CH = 16  # pairs per weight DMA chunk
NCH = NPAIR // CH

_NC_CACHE = None
_SIM_RELU = False  # CoreSim has no Prelu; debug builds swap in Relu
_SKIP_SPLIT = False  # sim-only: skip the walrus single-wait workaround


def _build():
    global _NC_CACHE
    if _NC_CACHE is not None:
        return _NC_CACHE

    f32 = mybir.dt.float32
    f16 = mybir.dt.float16
    bf16 = mybir.dt.bfloat16

    nc = bass.Bass()
    xt1 = nc.declare_dram_parameter("xt1", [128, B], bf16, isOutput=False)
    w0t = nc.declare_dram_parameter("w0t", [128, NPAIR // 2, 128], bf16, isOutput=False)
    b0p = nc.declare_dram_parameter("b0p", [128, NPAIR], f32, isOutput=False)
    w1t = nc.declare_dram_parameter("w1t", [128, NPAIR, 128], bf16, isOutput=False)
    b1p = nc.declare_dram_parameter("b1p", [128, NPAIR], f32, isOutput=False)
    out = nc.declare_dram_parameter("out", [NPAIR, 128, B], f16, isOutput=True)

    with ExitStack() as ctx:
        tc = ctx.enter_context(tile.TileContext(nc))
        singles = ctx.enter_context(tc.tile_pool(name="singles", bufs=1))
        hpool = ctx.enter_context(tc.tile_pool(name="hpool", bufs=6))
        opool = ctx.enter_context(tc.tile_pool(name="opool", bufs=6))
        pspool = ctx.enter_context(tc.tile_pool(name="psum", bufs=4, space="PSUM"))

        # Input loads ride idle engines' DMA rings so the sync ring carries
        # ONLY the output stream: startup-critical tensors (xt, first weight
        # chunk, b0) go HWDGE-via-scalar (ACT is idle until the first Prelu);
        # bulk weight chunks + b1 go SWDGE-via-gpsimd (fully idle engine,
        # ~32-pair prefetch lead swallows the higher fixed latency).
        xt = singles.tile([128, B], bf16)
        nc.scalar.dma_start(out=xt[0:64], in_=xt1[0:64])
        nc.sync.dma_start(out=xt[64:128], in_=xt1[64:128])

        w0sb = singles.tile([128, NPAIR // 2, 128], bf16)
        w1sb = singles.tile([128, NPAIR, 128], bf16)

        bounds = [0, 4, 16, 32, 48, 64, 80, 96, 112, 128]

        def load_chunk(i, eng):
            lo, hi = bounds[i], bounds[i + 1]
            eng.dma_start(
                out=w0sb[:, lo // 2 : hi // 2, :], in_=w0t[:, lo // 2 : hi // 2, :]
            )
            eng.dma_start(out=w1sb[:, lo:hi, :], in_=w1t[:, lo:hi, :])

        # chunk 0 split: w0 (mm0-critical) on scalar, w1 on sync
        nc.scalar.dma_start(out=w0sb[:, 0:2, :], in_=w0t[:, 0:2, :])
        nc.sync.dma_start(out=w1sb[:, 0:4, :], in_=w1t[:, 0:4, :])
        b0sb = singles.tile([128, NPAIR], f32)
        nc.gpsimd.dma_start(out=b0sb, in_=b0p[:])
        b1sb = singles.tile([128, NPAIR], f32)
        nc.gpsimd.dma_start(out=b1sb, in_=b1p[:])
        load_chunk(1, nc.gpsimd)


        NSUP = NPAIR // 2
        next_chunk = 2
        hsbs = [None, None]
        for u in range(NSUP + 1):
            # keep weight loads ~32 pairs ahead of the consuming pairs
            while next_chunk < len(bounds) - 1 and bounds[next_chunk] < 2 * u + 32:
                load_chunk(next_chunk, nc.gpsimd)
                next_chunk += 1
            if u < NSUP:
                # layer-0: two pairs as concurrent row-tiles (rows 0-63 and
                # 64-127 of the PE array share the moving bus perfectly)
                hpsA = pspool.tile([128, B], f32, tag="ps", name=f"hps{2 * u}")
                hpsB = pspool.tile([128, B], f32, tag="ps", name=f"hps{2 * u + 1}")
                for nb in range(2):
                    s = bass.ts(nb, 512)
                    nc.tensor.matmul(
                        hpsA[:, s], w0sb[0:64, u, :], xt[0:64, s],
                        start=True, stop=True,
                    )
                    nc.tensor.matmul(
                        hpsB[:, s], w0sb[64:128, u, :], xt[64:128, s],
                        start=True, stop=True,
                    )
            if u >= 1:
                opss = []
                for p in (2 * u - 2, 2 * u - 1):
                    ops = pspool.tile([128, B], f32, tag="ps", name=f"ops{p}")
                    for nb in range(2):
                        s = bass.ts(nb, 512)
                        nc.tensor.matmul(
                            ops[:, s], w1sb[:, p, :], hsbs[p % 2][:, s],
                            start=True, stop=True,
                        )
                    opss.append(ops)
            if u < NSUP:
                for i, hps in enumerate((hpsA, hpsB)):
                    t = 2 * u + i
                    hsb_new = hpool.tile([128, B], bf16, tag="h", name=f"hsb{t}")
                    nc.scalar.activation(
                        out=hsb_new,
                        in_=hps,
                        func=mybir.ActivationFunctionType.Relu
                        if _SIM_RELU
                        else mybir.ActivationFunctionType.Prelu,
                        bias=b0sb[:, t : t + 1],
                        scale=1.0,
                        alpha=NEG_SLOPE,
                    )
                    hsbs[i] = hsb_new
            if u >= 1:
                for i, p in enumerate((2 * u - 2, 2 * u - 1)):
                    osb = opool.tile([128, B], f16, tag="o", name=f"osb{p}")
                    if p % 8 == 3 or p >= 126:
                        # 18/128 pairs split their layer-1 evacuation between
                        # the engines (different PSUM banks, same per-partition
                        # b1): DVE 708ns + ACT 590ns instead of DVE 1192ns.
                        # Finer-grained ACT/DVE balancing than whole-pair
                        # offloads and leaves no dead gap on the DVE.
                        nc.vector.tensor_scalar_add(
                            osb[:, 0:512], opss[i][:, 0:512], b1sb[:, p : p + 1]
                        )
                        nc.scalar.add(
                            osb[:, 512:1024], opss[i][:, 512:1024], b1sb[:, p : p + 1]
                        )
                    else:
                        nc.vector.tensor_scalar_add(osb, opss[i], b1sb[:, p : p + 1])
                    nc.sync.dma_start(out=out[p], in_=osb)

    if not _SKIP_SPLIT:
        _split_multi_waits(nc)
    _NC_CACHE = nc
    return nc


def _split_multi_waits(nc):
    """Walrus in this toolchain allows at most ONE semaphore wait per
    instruction (and zero on the fused fp32 LDWEIGHTS struct).  Hoist all
    but the last wait of any multi-wait instruction onto same-engine NoOp
    carriers inserted directly before it — semantically identical (engine
    queues are in-order) and each carrier holds a single wait."""
    import bass_rust

    n = 0
    for f in nc.m.functions:
        for bb in f.blocks:
            out_insts = []
            changed = False
            for inst in bb.instructions:
                si = inst.sync_info
                waits = list(si.on_wait) if si is not None and si.on_wait else []
                if len(waits) > 1:
                    changed = True
                    for w in waits[:-1]:
                        nop = bass_rust.InstNoOp(
                            name=f"{inst.name}-sw{n}", engine=inst.engine
                        )
                        n += 1
                        nop.sync_info = mybir.SyncInfo(on_wait=[w], on_update=[])
                        out_insts.append(nop)
                    inst.sync_info = mybir.SyncInfo(
                        on_wait=[waits[-1]],
                        on_update=list(si.on_update) if si.on_update else [],
                    )
                out_insts.append(inst)
            if changed:
                bb.instructions = out_insts
    return nc


def _prepare_in_maps(x, W0, b0, W1, b1):
    x = np.asarray(x, dtype=np.float32)
    xT = x.T.astype(bfloat16)
    xt1 = np.ascontiguousarray(np.concatenate([xT, xT], axis=0))  # (128, B)

    in_maps = []
    for c in range(N_CORES):
        sl = slice(c * GPC, (c + 1) * GPC)
        W0c = np.asarray(W0[sl], dtype=np.float32)  # (256, 64, 64) [g, j, k]
        W1c = np.asarray(W1[sl], dtype=np.float32)
        b0c = np.asarray(b0[sl], dtype=np.float32)  # (256, 64)
        b1c = np.asarray(b1[sl], dtype=np.float32)

        # w0t[64*(t%2)+k, t//2, q*64+j] = W0[2t+q, j, k]  (row-tile stack)
        w0k = W0c.transpose(2, 0, 1).reshape(IN_DIM, NPAIR, 128)
        w0 = np.ascontiguousarray(
            w0k.reshape(IN_DIM, NPAIR // 2, 2, 128)
            .transpose(2, 0, 1, 3)
            .reshape(128, NPAIR // 2, 128)
            .astype(bfloat16)
        )

        # w1t[q*64+k, t, q'*64+j] = W1[2t+q, j, k] iff q == q'
        w1k = W1c.transpose(2, 0, 1).reshape(D1, NPAIR, 2, D2)  # [k, t, q, j]
        w1 = np.zeros((2, D1, NPAIR, 2, D2), dtype=bfloat16)
        for q in range(2):
            w1[q, :, :, q, :] = w1k[:, :, q, :].astype(bfloat16)
        w1 = np.ascontiguousarray(w1.reshape(128, NPAIR, 128))

        b0pp = np.ascontiguousarray(b0c.reshape(NPAIR, 128).T)  # (128, NPAIR)
        b1pp = np.ascontiguousarray(b1c.reshape(NPAIR, 128).T)
        in_maps.append(
            {"xt1": xt1, "w0t": w0, "w1t": w1, "b0p": b0pp, "b1p": b1pp}
        )
    return in_maps


def _postprocess(results):
    outs = []
    for c in range(N_CORES):
        o = results[c]["out"]  # (NPAIR, 128, B) f16 = [t, q*64+j, b]
        o = (
            o.astype(np.float32)
            .reshape(NPAIR, 2, D2, B)
            .transpose(3, 0, 1, 2)
            .reshape(B, GPC, D2)
        )
        outs.append(o)
    return np.ascontiguousarray(np.concatenate(outs, axis=1))


def _run(inputs, trace=False):
    nc = _build()
    in_maps = _prepare_in_maps(**inputs)
    res = run_bass_kernel_spmd(
        nc, in_maps, core_ids=list(range(N_CORES)), trace=trace
    )
    return _postprocess(res.results), res


def kernel(x, W0, b0, W1, b1):
    out, _ = _run({"x": x, "W0": W0, "b0": b0, "W1": W1, "b1": b1})
    return out


# revision 20
# speedup vs baseline: 1.0173x; 1.0173x over previous
"""Grouped 2-layer MLP (ConvNN) Trainium2 kernel.

Math (per group g of SIZE=2048):
    h[b,g,:]   = LeakyReLU_0.2(W0[g] @ x[b] + b0[g])     (64 -> 64)
    out[b,g,:] = W1[g] @ h[b,g,:] + b1[g]                (64 -> 64)

Measured on the target axon-tunneled TRN2: 181.7us HW exec (baseline
was 500.3us), rel err 3.2e-3 (gate 2e-2).  Per-core engine busy at the
final shape: DVE 145.6us / ACT 143.8us (the joint PSUM-evacuation
floor — TRN2 matmul output must be fp32, so every one of the 33.6M
evacuated elements per core crosses PSUM->SBUF at 1 elem/lane/cycle),
PE ~137us, DMA ~108us.

Strategy (row-tiled L0, evacuation-balanced pipeline):
  - Shard the group axis over 8 cores (256 groups/core = 128 pairs of
    groups), fully independent, no collectives.
  - Per pair t the dataflow is
        mm0 (PE) -> hps (PSUM fp32) -> Prelu+b0 (ACT) -> hsb (SBUF bf16)
        mm1 (PE) -> ops (PSUM fp32) -> +b1 (DVE)      -> osb (SBUF f16) -> DMA
    With FD=1024 per-pair evacuation ops: ACT ~1.05us, DVE ~1.19us; 8 of
    128 pairs route the layer-1 bias-add to ACT so both engines sit at
    ~144us.  PSUM: 4 rotating [128,1024] fp32 tiles (2 banks) = all 8
    banks, hps/ops double-buffered.
  - Layer-0 runs TWO pairs concurrently as PE row-tiles: even pair on
    array rows 0-63, odd pair on rows 64-127 (x duplicated on both
    partition halves, per-pair W0 stationaries stacked likewise).  Each
    K=64 stream uses half the 256B/cycle moving bus, so the two streams
    coexist and L0 time halves.  The denser activity also keeps the HAM
    clock-gate warm (2.4GHz) for the whole run — without row-tiling this
    environment's PE stayed throttled at 1.2GHz (427ns vs 216ns per
    N=512 matmul) despite 99% occupancy.
  - Layer-1 stationary is a host-built 128x128 block-diagonal (2 groups
    per pair, K=128 uses the full bus); off-diagonal zeros kill cross
    terms and cost nothing since matmul time is N-driven.
  - PE instruction stream is software-pipelined (mm0 of pairs 2u,2u+1
    emitted before mm1 of pairs 2u-2,2u-1) so the PE never waits on ACT.
  - Weights are host-packed exactly in stationary layout and streamed in
    chunks (small first chunk for a fast start) ~32 pairs ahead of use:
    startup-critical loads on the scalar HWDGE ring, bulk on the gpsimd
    SWDGE ring, leaving the sync ring exclusively for the output stream.
  - bf16 matmul operands (fp8 fails the 2e-2 gate: even single-tensor
    e4m3 variants measure 1.9-3.5e-2), f16 output (halves write traffic)
    widened to fp32 on host.
"""

from contextlib import ExitStack

import numpy as np
from ml_dtypes import bfloat16

import concourse.bass as bass
import concourse.mybir as mybir
import concourse.tile as tile
from concourse.bass_utils import run_bass_kernel_spmd

B = 1024
IN_DIM = 64
SIZE = 2048
D1 = 64
D2 = 64
NEG_SLOPE = 0.2
N_CORES = 8
GPC = SIZE // N_CORES  # 256 groups per core
NPAIR = GPC // 2  # 128 group-pairs per core
CH = 16  # pairs per weight DMA chunk
NCH = NPAIR // CH

_NC_CACHE = None
_SIM_RELU = False  # CoreSim has no Prelu; debug builds swap in Relu
_SKIP_SPLIT = False  # sim-only: skip the walrus single-wait workaround


def _build():
    global _NC_CACHE
    if _NC_CACHE is not None:
        return _NC_CACHE

    f32 = mybir.dt.float32
    f16 = mybir.dt.float16
    bf16 = mybir.dt.bfloat16

    nc = bass.Bass()
    xt1 = nc.declare_dram_parameter("xt1", [128, B], bf16, isOutput=False)
    w0t = nc.declare_dram_parameter("w0t", [128, NPAIR // 2, 128], bf16, isOutput=False)
    b0p = nc.declare_dram_parameter("b0p", [128, NPAIR], f32, isOutput=False)
    w1t = nc.declare_dram_parameter("w1t", [128, NPAIR, 128], bf16, isOutput=False)
    b1p = nc.declare_dram_parameter("b1p", [128, NPAIR], f32, isOutput=False)
    out = nc.declare_dram_parameter("out", [NPAIR, 128, B], f16, isOutput=True)

    with ExitStack() as ctx:
        tc = ctx.enter_context(tile.TileContext(nc))
        singles = ctx.enter_context(tc.tile_pool(name="singles", bufs=1))
        hpool = ctx.enter_context(tc.tile_pool(name="hpool", bufs=6))
        opool = ctx.enter_context(tc.tile_pool(name="opool", bufs=6))
        pspool = ctx.enter_context(tc.tile_pool(name="psum", bufs=4, space="PSUM"))

        # Input loads ride idle engines' DMA rings so the sync ring carries
        # ONLY the output stream: startup-critical tensors (xt, first weight
        # chunk, b0) go HWDGE-via-scalar (ACT is idle until the first Prelu);
        # bulk weight chunks + b1 go SWDGE-via-gpsimd (fully idle engine,
        # ~32-pair prefetch lead swallows the higher fixed latency).
        # Warm-up during the DMA fill: dependency-free dummy matmuls on an
        # uninitialized scratch tile trip the HAM clock-gate to 2.4GHz and a
        # dummy Prelu pulls the ~2.7us ACT table load off the first real
        # pair's critical path.  Results land in the first hps ring slot and
        # are overwritten by the first real start=True matmul.
        wscr = singles.tile([64, 512], bf16)
        nc.gpsimd.memset(wscr, 0.0)
        warm = pspool.tile([128, 512], f32, tag="ps", name="warm", bufs=4)
        for _ in range(8):
            nc.tensor.matmul(warm, wscr[:, 0:128], wscr, start=True, stop=True)
        ascr = singles.tile([128, 8], f32)
        nc.gpsimd.memset(ascr, 0.0)
        nc.scalar.activation(
            out=ascr,
            in_=ascr,
            func=mybir.ActivationFunctionType.Relu
            if _SIM_RELU
            else mybir.ActivationFunctionType.Prelu,
            bias=0.0 if _SIM_RELU else ascr[:, 0:1],
            scale=1.0,
            alpha=NEG_SLOPE,
        )

        xt = singles.tile([128, B], bf16)
        nc.scalar.dma_start(out=xt, in_=xt1[:])

        w0sb = singles.tile([128, NPAIR // 2, 128], bf16)
        w1sb = singles.tile([128, NPAIR, 128], bf16)

        bounds = [0, 4, 16, 32, 48, 64, 80, 96, 112, 128]

        def load_chunk(i, eng):
            lo, hi = bounds[i], bounds[i + 1]
            eng.dma_start(
                out=w0sb[:, lo // 2 : hi // 2, :], in_=w0t[:, lo // 2 : hi // 2, :]
            )
            eng.dma_start(out=w1sb[:, lo:hi, :], in_=w1t[:, lo:hi, :])

        load_chunk(0, nc.scalar)
        b0sb = singles.tile([128, NPAIR], f32)
        nc.scalar.dma_start(out=b0sb, in_=b0p[:])
        b1sb = singles.tile([128, NPAIR], f32)
        nc.gpsimd.dma_start(out=b1sb, in_=b1p[:])
        load_chunk(1, nc.gpsimd)


        NSUP = NPAIR // 2
        next_chunk = 2
        hsbs = [None, None]
        for u in range(NSUP + 1):
            # keep weight loads ~32 pairs ahead of the consuming pairs
            while next_chunk < len(bounds) - 1 and bounds[next_chunk] < 2 * u + 32:
                load_chunk(next_chunk, nc.gpsimd)
                next_chunk += 1
            if u < NSUP:
                # layer-0: two pairs as concurrent row-tiles (rows 0-63 and
                # 64-127 of the PE array share the moving bus perfectly)
                hpsA = pspool.tile([128, B], f32, tag="ps", name=f"hps{2 * u}")
                hpsB = pspool.tile([128, B], f32, tag="ps", name=f"hps{2 * u + 1}")
                for nb in range(2):
                    s = bass.ts(nb, 512)
                    nc.tensor.matmul(
                        hpsA[:, s], w0sb[0:64, u, :], xt[0:64, s],
                        start=True, stop=True,
                    )
                    nc.tensor.matmul(
                        hpsB[:, s], w0sb[64:128, u, :], xt[64:128, s],
                        start=True, stop=True,
                    )
            if u >= 1:
                opss = []
                for p in (2 * u - 2, 2 * u - 1):
                    ops = pspool.tile([128, B], f32, tag="ps", name=f"ops{p}")
                    for nb in range(2):
                        s = bass.ts(nb, 512)
                        nc.tensor.matmul(
                            ops[:, s], w1sb[:, p, :], hsbs[p % 2][:, s],
                            start=True, stop=True,
                        )
                    opss.append(ops)
            if u < NSUP:
                for i, hps in enumerate((hpsA, hpsB)):
                    t = 2 * u + i
                    hsb_new = hpool.tile([128, B], bf16, tag="h", name=f"hsb{t}")
                    nc.scalar.activation(
                        out=hsb_new,
                        in_=hps,
                        func=mybir.ActivationFunctionType.Relu
                        if _SIM_RELU
                        else mybir.ActivationFunctionType.Prelu,
                        bias=b0sb[:, t : t + 1],
                        scale=1.0,
                        alpha=NEG_SLOPE,
                    )
                    hsbs[i] = hsb_new
            if u >= 1:
                for i, p in enumerate((2 * u - 2, 2 * u - 1)):
                    osb = opool.tile([128, B], f16, tag="o", name=f"osb{p}")
                    if p % 16 == 7:
                        # 8/128 pairs evacuate layer-1 on ACT (identity+bias)
                        # to balance ACT (~1.05us/op) vs DVE (~1.19us/op)
                        nc.scalar.add(osb, opss[i], b1sb[:, p : p + 1])
                    else:
                        nc.vector.tensor_scalar_add(osb, opss[i], b1sb[:, p : p + 1])
                    nc.sync.dma_start(out=out[p], in_=osb)

    if not _SKIP_SPLIT:
        _split_multi_waits(nc)
    _NC_CACHE = nc
    return nc


def _split_multi_waits(nc):
    """Walrus in this toolchain allows at most ONE semaphore wait per
    instruction (and zero on the fused fp32 LDWEIGHTS struct).  Hoist all
    but the last wait of any multi-wait instruction onto same-engine NoOp
    carriers inserted directly before it — semantically identical (engine
    queues are in-order) and each carrier holds a single wait."""
    import bass_rust

    n = 0
    for f in nc.m.functions:
        for bb in f.blocks:
            out_insts = []
            changed = False
            for inst in bb.instructions:
                si = inst.sync_info
                waits = list(si.on_wait) if si is not None and si.on_wait else []
                if len(waits) > 1:
                    changed = True
                    for w in waits[:-1]:
                        nop = bass_rust.InstNoOp(
                            name=f"{inst.name}-sw{n}", engine=inst.engine
                        )
                        n += 1
                        nop.sync_info = mybir.SyncInfo(on_wait=[w], on_update=[])
                        out_insts.append(nop)
                    inst.sync_info = mybir.SyncInfo(
                        on_wait=[waits[-1]],
                        on_update=list(si.on_update) if si.on_update else [],
                    )
                out_insts.append(inst)
            if changed:
                bb.instructions = out_insts
    return nc


def _prepare_in_maps(x, W0, b0, W1, b1):
    x = np.asarray(x, dtype=np.float32)
    xT = x.T.astype(bfloat16)
    xt1 = np.ascontiguousarray(np.concatenate([xT, xT], axis=0))  # (128, B)

    in_maps = []
    for c in range(N_CORES):
        sl = slice(c * GPC, (c + 1) * GPC)
        W0c = np.asarray(W0[sl], dtype=np.float32)  # (256, 64, 64) [g, j, k]
        W1c = np.asarray(W1[sl], dtype=np.float32)
        b0c = np.asarray(b0[sl], dtype=np.float32)  # (256, 64)
        b1c = np.asarray(b1[sl], dtype=np.float32)

        # w0t[64*(t%2)+k, t//2, q*64+j] = W0[2t+q, j, k]  (row-tile stack)
        w0k = W0c.transpose(2, 0, 1).reshape(IN_DIM, NPAIR, 128)
        w0 = np.ascontiguousarray(
            w0k.reshape(IN_DIM, NPAIR // 2, 2, 128)
            .transpose(2, 0, 1, 3)
            .reshape(128, NPAIR // 2, 128)
            .astype(bfloat16)
        )

        # w1t[q*64+k, t, q'*64+j] = W1[2t+q, j, k] iff q == q'
        w1k = W1c.transpose(2, 0, 1).reshape(D1, NPAIR, 2, D2)  # [k, t, q, j]
        w1 = np.zeros((2, D1, NPAIR, 2, D2), dtype=bfloat16)
        for q in range(2):
            w1[q, :, :, q, :] = w1k[:, :, q, :].astype(bfloat16)
        w1 = np.ascontiguousarray(w1.reshape(128, NPAIR, 128))

        b0pp = np.ascontiguousarray(b0c.reshape(NPAIR, 128).T)  # (128, NPAIR)
        b1pp = np.ascontiguousarray(b1c.reshape(NPAIR, 128).T)
        in_maps.append(
            {"xt1": xt1, "w0t": w0, "w1t": w1, "b0p": b0pp, "b1p": b1pp}
        )
    return in_maps


def _postprocess(results):
    outs = []
    for c in range(N_CORES):
        o = results[c]["out"]  # (NPAIR, 128, B) f16 = [t, q*64+j, b]
        o = (
            o.astype(np.float32)
            .reshape(NPAIR, 2, D2, B)
            .transpose(3, 0, 1, 2)
            .reshape(B, GPC, D2)
        )
        outs.append(o)
    return np.ascontiguousarray(np.concatenate(outs, axis=1))


def _run(inputs, trace=False):
    nc = _build()
    in_maps = _prepare_in_maps(**inputs)
    res = run_bass_kernel_spmd(
        nc, in_maps, core_ids=list(range(N_CORES)), trace=trace
    )
    return _postprocess(res.results), res


def kernel(x, W0, b0, W1, b1):
    out, _ = _run({"x": x, "W0": W0, "b0": b0, "W1": W1, "b1": b1})
    return out


# revision 21
# speedup vs baseline: 1.0181x; 1.0008x over previous
"""Grouped 2-layer MLP (ConvNN) Trainium2 kernel.

Math (per group g of SIZE=2048):
    h[b,g,:]   = LeakyReLU_0.2(W0[g] @ x[b] + b0[g])     (64 -> 64)
    out[b,g,:] = W1[g] @ h[b,g,:] + b1[g]                (64 -> 64)

Measured on the target axon-tunneled TRN2: 181.7us HW exec (baseline
was 500.3us), rel err 3.2e-3 (gate 2e-2).  Per-core engine busy at the
final shape: DVE 145.6us / ACT 143.8us (the joint PSUM-evacuation
floor — TRN2 matmul output must be fp32, so every one of the 33.6M
evacuated elements per core crosses PSUM->SBUF at 1 elem/lane/cycle),
PE ~137us, DMA ~108us.

Strategy (row-tiled L0, evacuation-balanced pipeline):
  - Shard the group axis over 8 cores (256 groups/core = 128 pairs of
    groups), fully independent, no collectives.
  - Per pair t the dataflow is
        mm0 (PE) -> hps (PSUM fp32) -> Prelu+b0 (ACT) -> hsb (SBUF bf16)
        mm1 (PE) -> ops (PSUM fp32) -> +b1 (DVE)      -> osb (SBUF f16) -> DMA
    With FD=1024 per-pair evacuation ops: ACT ~1.05us, DVE ~1.19us; 8 of
    128 pairs route the layer-1 bias-add to ACT so both engines sit at
    ~144us.  PSUM: 4 rotating [128,1024] fp32 tiles (2 banks) = all 8
    banks, hps/ops double-buffered.
  - Layer-0 runs TWO pairs concurrently as PE row-tiles: even pair on
    array rows 0-63, odd pair on rows 64-127 (x duplicated on both
    partition halves, per-pair W0 stationaries stacked likewise).  Each
    K=64 stream uses half the 256B/cycle moving bus, so the two streams
    coexist and L0 time halves.  The denser activity also keeps the HAM
    clock-gate warm (2.4GHz) for the whole run — without row-tiling this
    environment's PE stayed throttled at 1.2GHz (427ns vs 216ns per
    N=512 matmul) despite 99% occupancy.
  - Layer-1 stationary is a host-built 128x128 block-diagonal (2 groups
    per pair, K=128 uses the full bus); off-diagonal zeros kill cross
    terms and cost nothing since matmul time is N-driven.
  - PE instruction stream is software-pipelined (mm0 of pairs 2u,2u+1
    emitted before mm1 of pairs 2u-2,2u-1) so the PE never waits on ACT.
  - Weights are host-packed exactly in stationary layout and streamed in
    chunks (small first chunk for a fast start) ~32 pairs ahead of use:
    startup-critical loads on the scalar HWDGE ring, bulk on the gpsimd
    SWDGE ring, leaving the sync ring exclusively for the output stream.
  - bf16 matmul operands (fp8 fails the 2e-2 gate: even single-tensor
    e4m3 variants measure 1.9-3.5e-2), f16 output (halves write traffic)
    widened to fp32 on host.
"""

from contextlib import ExitStack

import numpy as np
from ml_dtypes import bfloat16

import concourse.bass as bass
import concourse.mybir as mybir
import concourse.tile as tile
from concourse.bass_utils import run_bass_kernel_spmd

B = 1024
IN_DIM = 64
SIZE = 2048
D1 = 64
D2 = 64
NEG_SLOPE = 0.2
N_CORES = 8
GPC = SIZE // N_CORES  # 256 groups per core
NPAIR = GPC // 2  # 128 group-pairs per core
CH = 16  # pairs per weight DMA chunk
NCH = NPAIR // CH

_NC_CACHE = None
_SIM_RELU = False  # CoreSim has no Prelu; debug builds swap in Relu
_SKIP_SPLIT = False  # sim-only: skip the walrus single-wait workaround


def _build():
    global _NC_CACHE
    if _NC_CACHE is not None:
        return _NC_CACHE

    f32 = mybir.dt.float32
    f16 = mybir.dt.float16
    bf16 = mybir.dt.bfloat16

    nc = bass.Bass()
    xt1 = nc.declare_dram_parameter("xt1", [128, B], bf16, isOutput=False)
    w0t = nc.declare_dram_parameter("w0t", [128, NPAIR // 2, 128], bf16, isOutput=False)
    b0p = nc.declare_dram_parameter("b0p", [128, NPAIR], f32, isOutput=False)
    w1t = nc.declare_dram_parameter("w1t", [128, NPAIR, 128], bf16, isOutput=False)
    b1p = nc.declare_dram_parameter("b1p", [128, NPAIR], f32, isOutput=False)
    out = nc.declare_dram_parameter("out", [NPAIR, 128, B], f16, isOutput=True)

    with ExitStack() as ctx:
        tc = ctx.enter_context(tile.TileContext(nc))
        singles = ctx.enter_context(tc.tile_pool(name="singles", bufs=1))
        hpool = ctx.enter_context(tc.tile_pool(name="hpool", bufs=6))
        opool = ctx.enter_context(tc.tile_pool(name="opool", bufs=6))
        pspool = ctx.enter_context(tc.tile_pool(name="psum", bufs=4, space="PSUM"))

        # Input loads ride idle engines' DMA rings so the sync ring carries
        # ONLY the output stream: startup-critical tensors (xt, first weight
        # chunk, b0) go HWDGE-via-scalar (ACT is idle until the first Prelu);
        # bulk weight chunks + b1 go SWDGE-via-gpsimd (fully idle engine,
        # ~32-pair prefetch lead swallows the higher fixed latency).
        xt = singles.tile([128, B], bf16)
        nc.scalar.dma_start(out=xt, in_=xt1[:])

        w0sb = singles.tile([128, NPAIR // 2, 128], bf16)
        w1sb = singles.tile([128, NPAIR, 128], bf16)

        bounds = [0, 4, 16, 32, 48, 64, 80, 96, 112, 128]

        def load_chunk(i, eng):
            lo, hi = bounds[i], bounds[i + 1]
            eng.dma_start(
                out=w0sb[:, lo // 2 : hi // 2, :], in_=w0t[:, lo // 2 : hi // 2, :]
            )
            eng.dma_start(out=w1sb[:, lo:hi, :], in_=w1t[:, lo:hi, :])

        load_chunk(0, nc.scalar)
        b0sb = singles.tile([128, NPAIR], f32)
        nc.scalar.dma_start(out=b0sb, in_=b0p[:])
        b1sb = singles.tile([128, NPAIR], f32)
        nc.gpsimd.dma_start(out=b1sb, in_=b1p[:])
        load_chunk(1, nc.gpsimd)


        NSUP = NPAIR // 2
        next_chunk = 2
        hsbs = [None, None]
        for u in range(NSUP + 1):
            # keep weight loads ~32 pairs ahead of the consuming pairs
            while next_chunk < len(bounds) - 1 and bounds[next_chunk] < 2 * u + 32:
                load_chunk(next_chunk, nc.gpsimd)
                next_chunk += 1
            if u < NSUP:
                # layer-0: two pairs as concurrent row-tiles (rows 0-63 and
                # 64-127 of the PE array share the moving bus perfectly)
                hpsA = pspool.tile([128, B], f32, tag="ps", name=f"hps{2 * u}")
                hpsB = pspool.tile([128, B], f32, tag="ps", name=f"hps{2 * u + 1}")
                for nb in range(2):
                    s = bass.ts(nb, 512)
                    nc.tensor.matmul(
                        hpsA[:, s], w0sb[0:64, u, :], xt[0:64, s],
                        start=True, stop=True,
                    )
                    nc.tensor.matmul(
                        hpsB[:, s], w0sb[64:128, u, :], xt[64:128, s],
                        start=True, stop=True,
                    )
            if u >= 1:
                opss = []
                for p in (2 * u - 2, 2 * u - 1):
                    ops = pspool.tile([128, B], f32, tag="ps", name=f"ops{p}")
                    for nb in range(2):
                        s = bass.ts(nb, 512)
                        nc.tensor.matmul(
                            ops[:, s], w1sb[:, p, :], hsbs[p % 2][:, s],
                            start=True, stop=True,
                        )
                    opss.append(ops)
            if u < NSUP:
                for i, hps in enumerate((hpsA, hpsB)):
                    t = 2 * u + i
                    hsb_new = hpool.tile([128, B], bf16, tag="h", name=f"hsb{t}")
                    nc.scalar.activation(
                        out=hsb_new,
                        in_=hps,
                        func=mybir.ActivationFunctionType.Relu
                        if _SIM_RELU
                        else mybir.ActivationFunctionType.Prelu,
                        bias=b0sb[:, t : t + 1],
                        scale=1.0,
                        alpha=NEG_SLOPE,
                    )
                    hsbs[i] = hsb_new
            if u >= 1:
                for i, p in enumerate((2 * u - 2, 2 * u - 1)):
                    osb = opool.tile([128, B], f16, tag="o", name=f"osb{p}")
                    if p % 16 == 7:
                        # 8/128 pairs evacuate layer-1 on ACT (identity+bias)
                        # to balance ACT (~1.05us/op) vs DVE (~1.19us/op)
                        nc.scalar.add(osb, opss[i], b1sb[:, p : p + 1])
                    else:
                        nc.vector.tensor_scalar_add(osb, opss[i], b1sb[:, p : p + 1])
                    nc.sync.dma_start(out=out[p], in_=osb)

    if not _SKIP_SPLIT:
        _split_multi_waits(nc)
    _NC_CACHE = nc
    return nc


def _split_multi_waits(nc):
    """Walrus in this toolchain allows at most ONE semaphore wait per
    instruction (and zero on the fused fp32 LDWEIGHTS struct).  Hoist all
    but the last wait of any multi-wait instruction onto same-engine NoOp
    carriers inserted directly before it — semantically identical (engine
    queues are in-order) and each carrier holds a single wait."""
    import bass_rust

    n = 0
    for f in nc.m.functions:
        for bb in f.blocks:
            out_insts = []
            changed = False
            for inst in bb.instructions:
                si = inst.sync_info
                waits = list(si.on_wait) if si is not None and si.on_wait else []
                if len(waits) > 1:
                    changed = True
                    for w in waits[:-1]:
                        nop = bass_rust.InstNoOp(
                            name=f"{inst.name}-sw{n}", engine=inst.engine
                        )
                        n += 1
                        nop.sync_info = mybir.SyncInfo(on_wait=[w], on_update=[])
                        out_insts.append(nop)
                    inst.sync_info = mybir.SyncInfo(
                        on_wait=[waits[-1]],
                        on_update=list(si.on_update) if si.on_update else [],
                    )
                out_insts.append(inst)
            if changed:
                bb.instructions = out_insts
    return nc


def _prepare_in_maps(x, W0, b0, W1, b1):
    x = np.asarray(x, dtype=np.float32)
    xT = x.T.astype(bfloat16)
    xt1 = np.ascontiguousarray(np.concatenate([xT, xT], axis=0))  # (128, B)

    in_maps = []
    for c in range(N_CORES):
        sl = slice(c * GPC, (c + 1) * GPC)
        W0c = np.asarray(W0[sl], dtype=np.float32)  # (256, 64, 64) [g, j, k]
        W1c = np.asarray(W1[sl], dtype=np.float32)
        b0c = np.asarray(b0[sl], dtype=np.float32)  # (256, 64)
        b1c = np.asarray(b1[sl], dtype=np.float32)

        # w0t[64*(t%2)+k, t//2, q*64+j] = W0[2t+q, j, k]  (row-tile stack)
        w0k = W0c.transpose(2, 0, 1).reshape(IN_DIM, NPAIR, 128)
        w0 = np.ascontiguousarray(
            w0k.reshape(IN_DIM, NPAIR // 2, 2, 128)
            .transpose(2, 0, 1, 3)
            .reshape(128, NPAIR // 2, 128)
            .astype(bfloat16)
        )

        # w1t[q*64+k, t, q'*64+j] = W1[2t+q, j, k] iff q == q'
        w1k = W1c.transpose(2, 0, 1).reshape(D1, NPAIR, 2, D2)  # [k, t, q, j]
        w1 = np.zeros((2, D1, NPAIR, 2, D2), dtype=bfloat16)
        for q in range(2):
            w1[q, :, :, q, :] = w1k[:, :, q, :].astype(bfloat16)
        w1 = np.ascontiguousarray(w1.reshape(128, NPAIR, 128))

        b0pp = np.ascontiguousarray(b0c.reshape(NPAIR, 128).T)  # (128, NPAIR)
        b1pp = np.ascontiguousarray(b1c.reshape(NPAIR, 128).T)
        in_maps.append(
            {"xt1": xt1, "w0t": w0, "w1t": w1, "b0p": b0pp, "b1p": b1pp}
        )
    return in_maps


def _postprocess(results):
    outs = []
    for c in range(N_CORES):
        o = results[c]["out"]  # (NPAIR, 128, B) f16 = [t, q*64+j, b]
        o = (
            o.astype(np.float32)
            .reshape(NPAIR, 2, D2, B)
            .transpose(3, 0, 1, 2)
            .reshape(B, GPC, D2)
        )
        outs.append(o)
    return np.ascontiguousarray(np.concatenate(outs, axis=1))


def _run(inputs, trace=False):
    nc = _build()
    in_maps = _prepare_in_maps(**inputs)
    res = run_bass_kernel_spmd(
        nc, in_maps, core_ids=list(range(N_CORES)), trace=trace
    )
    return _postprocess(res.results), res


def kernel(x, W0, b0, W1, b1):
    out, _ = _run({"x": x, "W0": W0, "b0": b0, "W1": W1, "b1": b1})
    return out


# revision 23
# speedup vs baseline: 1.0268x; 1.0085x over previous
"""Grouped 2-layer MLP (ConvNN) Trainium2 kernel.

Math (per group g of SIZE=2048):
    h[b,g,:]   = LeakyReLU_0.2(W0[g] @ x[b] + b0[g])     (64 -> 64)
    out[b,g,:] = W1[g] @ h[b,g,:] + b1[g]                (64 -> 64)

Measured on the target axon-tunneled TRN2: 181.7us HW exec (baseline
was 500.3us), rel err 3.2e-3 (gate 2e-2).  Per-core engine busy at the
final shape: DVE 145.6us / ACT 143.8us (the joint PSUM-evacuation
floor — TRN2 matmul output must be fp32, so every one of the 33.6M
evacuated elements per core crosses PSUM->SBUF at 1 elem/lane/cycle),
PE ~137us, DMA ~108us.

Strategy (row-tiled L0, evacuation-balanced pipeline):
  - Shard the group axis over 8 cores (256 groups/core = 128 pairs of
    groups), fully independent, no collectives.
  - Per pair t the dataflow is
        mm0 (PE) -> hps (PSUM fp32) -> Prelu+b0 (ACT) -> hsb (SBUF bf16)
        mm1 (PE) -> ops (PSUM fp32) -> +b1 (DVE)      -> osb (SBUF f16) -> DMA
    With FD=1024 per-pair evacuation ops: ACT ~1.05us, DVE ~1.19us; 8 of
    128 pairs route the layer-1 bias-add to ACT so both engines sit at
    ~144us.  PSUM: 4 rotating [128,1024] fp32 tiles (2 banks) = all 8
    banks, hps/ops double-buffered.
  - Layer-0 runs TWO pairs concurrently as PE row-tiles: even pair on
    array rows 0-63, odd pair on rows 64-127 (x duplicated on both
    partition halves, per-pair W0 stationaries stacked likewise).  Each
    K=64 stream uses half the 256B/cycle moving bus, so the two streams
    coexist and L0 time halves.  The denser activity also keeps the HAM
    clock-gate warm (2.4GHz) for the whole run — without row-tiling this
    environment's PE stayed throttled at 1.2GHz (427ns vs 216ns per
    N=512 matmul) despite 99% occupancy.
  - Layer-1 stationary is a host-built 128x128 block-diagonal (2 groups
    per pair, K=128 uses the full bus); off-diagonal zeros kill cross
    terms and cost nothing since matmul time is N-driven.
  - PE instruction stream is software-pipelined (mm0 of pairs 2u,2u+1
    emitted before mm1 of pairs 2u-2,2u-1) so the PE never waits on ACT.
  - Weights are host-packed exactly in stationary layout and streamed in
    chunks (small first chunk for a fast start) ~32 pairs ahead of use:
    startup-critical loads on the scalar HWDGE ring, bulk on the gpsimd
    SWDGE ring, leaving the sync ring exclusively for the output stream.
  - bf16 matmul operands (fp8 fails the 2e-2 gate: even single-tensor
    e4m3 variants measure 1.9-3.5e-2), f16 output (halves write traffic)
    widened to fp32 on host.
"""

from contextlib import ExitStack

import numpy as np
from ml_dtypes import bfloat16

import concourse.bass as bass
import concourse.mybir as mybir
import concourse.tile as tile
from concourse.bass_utils import run_bass_kernel_spmd

B = 1024
IN_DIM = 64
SIZE = 2048
D1 = 64
D2 = 64
NEG_SLOPE = 0.2
N_CORES = 8
GPC = SIZE // N_CORES  # 256 groups per core
NPAIR = GPC // 2  # 128 group-pairs per core
CH = 16  # pairs per weight DMA chunk
NCH = NPAIR // CH

_NC_CACHE = None
_SIM_RELU = False  # CoreSim has no Prelu; debug builds swap in Relu
_SKIP_SPLIT = False  # sim-only: skip the walrus single-wait workaround


def _build():
    global _NC_CACHE
    if _NC_CACHE is not None:
        return _NC_CACHE

    f32 = mybir.dt.float32
    f16 = mybir.dt.float16
    bf16 = mybir.dt.bfloat16

    nc = bass.Bass()
    xt1 = nc.declare_dram_parameter("xt1", [128, B], bf16, isOutput=False)
    w0t = nc.declare_dram_parameter("w0t", [128, NPAIR // 2, 128], bf16, isOutput=False)
    b0p = nc.declare_dram_parameter("b0p", [128, NPAIR], f32, isOutput=False)
    w1t = nc.declare_dram_parameter("w1t", [128, NPAIR, 128], bf16, isOutput=False)
    b1p = nc.declare_dram_parameter("b1p", [128, NPAIR], f32, isOutput=False)
    out = nc.declare_dram_parameter("out", [NPAIR, 128, B], f16, isOutput=True)

    with ExitStack() as ctx:
        tc = ctx.enter_context(tile.TileContext(nc))
        singles = ctx.enter_context(tc.tile_pool(name="singles", bufs=1))
        hpool = ctx.enter_context(tc.tile_pool(name="hpool", bufs=6))
        opool = ctx.enter_context(tc.tile_pool(name="opool", bufs=6))
        pspool = ctx.enter_context(tc.tile_pool(name="psum", bufs=4, space="PSUM"))

        # Input loads ride idle engines' DMA rings so the sync ring carries
        # ONLY the output stream: startup-critical tensors (xt, first weight
        # chunk, b0) go HWDGE-via-scalar (ACT is idle until the first Prelu);
        # bulk weight chunks + b1 go SWDGE-via-gpsimd (fully idle engine,
        # ~32-pair prefetch lead swallows the higher fixed latency).
        xt = singles.tile([128, B], bf16)
        nc.scalar.dma_start(out=xt, in_=xt1[:])

        w0sb = singles.tile([128, NPAIR // 2, 128], bf16)
        w1sb = singles.tile([128, NPAIR, 128], bf16)

        bounds = [0, 4, 16, 32, 48, 64, 80, 96, 112, 128]

        def load_chunk(i, eng):
            lo, hi = bounds[i], bounds[i + 1]
            eng.dma_start(
                out=w0sb[:, lo // 2 : hi // 2, :], in_=w0t[:, lo // 2 : hi // 2, :]
            )
            eng.dma_start(out=w1sb[:, lo:hi, :], in_=w1t[:, lo:hi, :])

        load_chunk(0, nc.scalar)
        b0sb = singles.tile([128, NPAIR], f32)
        nc.scalar.dma_start(out=b0sb, in_=b0p[:])
        b1sb = singles.tile([128, NPAIR], f32)
        nc.gpsimd.dma_start(out=b1sb, in_=b1p[:])
        load_chunk(1, nc.gpsimd)


        NSUP = NPAIR // 2
        next_chunk = 2
        hsbs = [None, None]
        for u in range(NSUP + 1):
            # keep weight loads ~32 pairs ahead of the consuming pairs
            while next_chunk < len(bounds) - 1 and bounds[next_chunk] < 2 * u + 32:
                load_chunk(next_chunk, nc.gpsimd)
                next_chunk += 1
            # PE block interleave: mm0 chunk s of both row-tiled pairs, then
            # mm1s of one older pair — feeds the DVE ~450ns earlier each
            # iteration (closing the idle gap behind ACT-offloaded pairs)
            # while the Prelu of this iteration's pairs still gets most of an
            # iteration of grace before its mm1 consumer.
            def emit_mm1(p):
                ops = pspool.tile([128, B], f32, tag="ps", name=f"ops{p}")
                for nb in range(2):
                    s = bass.ts(nb, 512)
                    nc.tensor.matmul(
                        ops[:, s], w1sb[:, p, :], hsbs[p % 2][:, s],
                        start=True, stop=True,
                    )
                return ops

            if u < NSUP:
                # layer-0: two pairs as concurrent row-tiles (rows 0-63 and
                # 64-127 of the PE array share the moving bus perfectly)
                hpsA = pspool.tile([128, B], f32, tag="ps", name=f"hps{2 * u}")
                hpsB = pspool.tile([128, B], f32, tag="ps", name=f"hps{2 * u + 1}")
            opss = []
            for nb in range(2):
                s = bass.ts(nb, 512)
                if u < NSUP:
                    nc.tensor.matmul(
                        hpsA[:, s], w0sb[0:64, u, :], xt[0:64, s],
                        start=True, stop=True,
                    )
                    nc.tensor.matmul(
                        hpsB[:, s], w0sb[64:128, u, :], xt[64:128, s],
                        start=True, stop=True,
                    )
                if u >= 1:
                    opss.append(emit_mm1(2 * u - 2 + nb))
            if u < NSUP:
                for i, hps in enumerate((hpsA, hpsB)):
                    t = 2 * u + i
                    hsb_new = hpool.tile([128, B], bf16, tag="h", name=f"hsb{t}")
                    nc.scalar.activation(
                        out=hsb_new,
                        in_=hps,
                        func=mybir.ActivationFunctionType.Relu
                        if _SIM_RELU
                        else mybir.ActivationFunctionType.Prelu,
                        bias=b0sb[:, t : t + 1],
                        scale=1.0,
                        alpha=NEG_SLOPE,
                    )
                    hsbs[i] = hsb_new
            if u >= 1:
                for i, p in enumerate((2 * u - 2, 2 * u - 1)):
                    osb = opool.tile([128, B], f16, tag="o", name=f"osb{p}")
                    if p % 16 == 7:
                        # 8/128 pairs evacuate layer-1 on ACT (identity+bias)
                        # to balance ACT (~1.05us/op) vs DVE (~1.19us/op)
                        nc.scalar.add(osb, opss[i], b1sb[:, p : p + 1])
                    else:
                        nc.vector.tensor_scalar_add(osb, opss[i], b1sb[:, p : p + 1])
                    nc.sync.dma_start(out=out[p], in_=osb)

    if not _SKIP_SPLIT:
        _split_multi_waits(nc)
    _NC_CACHE = nc
    return nc


def _split_multi_waits(nc):
    """Walrus in this toolchain allows at most ONE semaphore wait per
    instruction (and zero on the fused fp32 LDWEIGHTS struct).  Hoist all
    but the last wait of any multi-wait instruction onto same-engine NoOp
    carriers inserted directly before it — semantically identical (engine
    queues are in-order) and each carrier holds a single wait."""
    import bass_rust

    n = 0
    for f in nc.m.functions:
        for bb in f.blocks:
            out_insts = []
            changed = False
            for inst in bb.instructions:
                si = inst.sync_info
                waits = list(si.on_wait) if si is not None and si.on_wait else []
                if len(waits) > 1:
                    changed = True
                    for w in waits[:-1]:
                        nop = bass_rust.InstNoOp(
                            name=f"{inst.name}-sw{n}", engine=inst.engine
                        )
                        n += 1
                        nop.sync_info = mybir.SyncInfo(on_wait=[w], on_update=[])
                        out_insts.append(nop)
                    inst.sync_info = mybir.SyncInfo(
                        on_wait=[waits[-1]],
                        on_update=list(si.on_update) if si.on_update else [],
                    )
                out_insts.append(inst)
            if changed:
                bb.instructions = out_insts
    return nc


def _prepare_in_maps(x, W0, b0, W1, b1):
    x = np.asarray(x, dtype=np.float32)
    xT = x.T.astype(bfloat16)
    xt1 = np.ascontiguousarray(np.concatenate([xT, xT], axis=0))  # (128, B)

    in_maps = []
    for c in range(N_CORES):
        sl = slice(c * GPC, (c + 1) * GPC)
        W0c = np.asarray(W0[sl], dtype=np.float32)  # (256, 64, 64) [g, j, k]
        W1c = np.asarray(W1[sl], dtype=np.float32)
        b0c = np.asarray(b0[sl], dtype=np.float32)  # (256, 64)
        b1c = np.asarray(b1[sl], dtype=np.float32)

        # w0t[64*(t%2)+k, t//2, q*64+j] = W0[2t+q, j, k]  (row-tile stack)
        w0k = W0c.transpose(2, 0, 1).reshape(IN_DIM, NPAIR, 128)
        w0 = np.ascontiguousarray(
            w0k.reshape(IN_DIM, NPAIR // 2, 2, 128)
            .transpose(2, 0, 1, 3)
            .reshape(128, NPAIR // 2, 128)
            .astype(bfloat16)
        )

        # w1t[q*64+k, t, q'*64+j] = W1[2t+q, j, k] iff q == q'
        w1k = W1c.transpose(2, 0, 1).reshape(D1, NPAIR, 2, D2)  # [k, t, q, j]
        w1 = np.zeros((2, D1, NPAIR, 2, D2), dtype=bfloat16)
        for q in range(2):
            w1[q, :, :, q, :] = w1k[:, :, q, :].astype(bfloat16)
        w1 = np.ascontiguousarray(w1.reshape(128, NPAIR, 128))

        b0pp = np.ascontiguousarray(b0c.reshape(NPAIR, 128).T)  # (128, NPAIR)
        b1pp = np.ascontiguousarray(b1c.reshape(NPAIR, 128).T)
        in_maps.append(
            {"xt1": xt1, "w0t": w0, "w1t": w1, "b0p": b0pp, "b1p": b1pp}
        )
    return in_maps


def _postprocess(results):
    outs = []
    for c in range(N_CORES):
        o = results[c]["out"]  # (NPAIR, 128, B) f16 = [t, q*64+j, b]
        o = (
            o.astype(np.float32)
            .reshape(NPAIR, 2, D2, B)
            .transpose(3, 0, 1, 2)
            .reshape(B, GPC, D2)
        )
        outs.append(o)
    return np.ascontiguousarray(np.concatenate(outs, axis=1))


def _run(inputs, trace=False):
    nc = _build()
    in_maps = _prepare_in_maps(**inputs)
    res = run_bass_kernel_spmd(
        nc, in_maps, core_ids=list(range(N_CORES)), trace=trace
    )
    return _postprocess(res.results), res


def kernel(x, W0, b0, W1, b1):
    out, _ = _run({"x": x, "W0": W0, "b0": b0, "W1": W1, "b1": b1})
    return out
